# revision 13
# baseline (speedup 1.0000x reference)
"""Fp8 per-token/per-channel quantized linear for Trainium2, 8 NeuronCores.

Computation (matches the jax reference):
    amax[m]  = max_k |x[m, k]|                       (x is bf16)
    xs[m]    = max(amax, 1e-10) / 448
    x_q      = e4m3fn_round(x / xs)                  (values up to +-448)
    out      = bf16((x_q @ W^T) * xs * w_scales) + bf16(bias)

Mapping to TRN2 hardware:
  * TRN's fp8 E4M3 saturates at +-240, so we quantize at HALF scale
    (factor folded into the output scale; exact on fp8's power-of-2 grid).
  * Sharding: row-parallel over M (8 cores x 1024 rows of x each); the full
    fp8-re-encoded weight streams through every core.
  * x is read from DRAM exactly once, via the DMA XBAR transpose
    (dma_start_transpose, SP ring only -- it corrupts data on the ACT ring)
    straight into the [k_lo, k_sub, m] layout the DoubleRow GEMM wants.
    There is no row-major x load at all: amax comes from the transposed
    pieces via a DVE abs_max tree + GpSimd partition_all_reduce(max), which
    also yields the quant scale already partition-broadcast (invb) with no
    DRAM round trip.  Only the output scale xs needs a tiny [1,128] ->
    DRAM -> [128,1] bounce.
  * Quantize: DVE multiplies piece A (16 ksubs), GpSimd piece B, fp8 out.
  * Ring budget (~125 GB/s per ring, 2 HWDGE rings): SP carries transposes
    (~8MB effective) + outputs (8MB) + scale bounces; ACT carries weights
    (16MB) + ws/bias block broadcasts.  Early weight slabs w1-w3 are split
    across both rings.  The GEMM walks (nb, mt) in rectangle-grow order so
    the early phase needs only a small corner of x-tiles x w-slabs.
  * Output stage fused: DVE scalar_tensor_tensor (psum*xs)*ws -> bf16,
    GpSimd adds host-precast bf16 bias, SP-ring DMA out.
  * PE runs ONLY the 1024 fp8 DoubleRow matmuls (k=256, n=512 each) with
    all 8 PSUM banks in flight.
"""

import os
import numpy as np
import ml_dtypes
from contextlib import ExitStack

import concourse.bass as bass
import concourse.bacc as bacc
import concourse.tile as tile
from concourse import mybir, bass_isa
from concourse.bass_utils import run_bass_kernel_spmd

P = 128
M, K, N = 8192, 4096, 4096
NCORES = 8
M_SHARD = M // NCORES          # 1024 rows of x per core
M_TILES = M_SHARD // P         # 8
K_SUBS = K // P                # 32
KH = K_SUBS // 2               # 16 ksubs per transpose/quantize piece
K_SUPERS = K // (2 * P)        # 16 (DoubleRow consumes 256 rows of K)
N_BLK = 512
N_BLKS = N // N_BLK            # 8

FP8 = mybir.dt.float8e4
F32 = mybir.dt.float32
BF16 = mybir.dt.bfloat16

_PROGRAM_CACHE = {}


def _gemm_order():
    """Rectangle-grow (staircase) enumeration of (nb, mt), mt-biased 4:1."""
    order = [(0, 0)]
    nm, nn = 1, 1
    while nm < M_TILES or nn < N_BLKS:
        if nm < M_TILES and (nm < 4 * nn or nn == N_BLKS):
            order.extend((nb, nm) for nb in range(nn))
            nm += 1
        else:
            order.extend((nn, mt) for mt in range(nm))
            nn += 1
    return order


def _build_program():
    nc = bacc.Bacc(None, target_bir_lowering=False)

    x_d = nc.declare_dram_parameter("x", [M_SHARD, K], BF16, isOutput=False)
    # host layout: wt[nb, p, ksub, n] = weight[nb*512 + n, ksub*128 + p], fp8
    wt_d = nc.declare_dram_parameter("wt", [N_BLKS, P, K_SUBS, N_BLK], FP8, isOutput=False)
    ws_d = nc.declare_dram_parameter("ws", [N], F32, isOutput=False)
    bias_d = nc.declare_dram_parameter("bias", [N], BF16, isOutput=False)
    out_d = nc.declare_dram_parameter("out", [M_SHARD, N], BF16, isOutput=True)

    xs_scr = nc.dram_tensor("xs_scratch", [M_TILES, P], F32, kind="Internal")

    x_ap = x_d[:]
    wt_ap = wt_d[:]
    out_ap = out_d[:]

    with tile.TileContext(nc) as tc, ExitStack() as ctx:
        xTpoolA = ctx.enter_context(tc.tile_pool(name="xTpoolA", bufs=3))
        xTpoolB = ctx.enter_context(tc.tile_pool(name="xTpoolB", bufs=3))
        xqpoolA = ctx.enter_context(tc.tile_pool(name="xqpoolA", bufs=M_TILES))
        xqpoolB = ctx.enter_context(tc.tile_pool(name="xqpoolB", bufs=M_TILES))
        wpool = ctx.enter_context(tc.tile_pool(name="wpool", bufs=4))
        tpabsA = ctx.enter_context(tc.tile_pool(name="tpabsA", bufs=1))
        tpabsB = ctx.enter_context(tc.tile_pool(name="tpabsB", bufs=1))
        tp8a = ctx.enter_context(tc.tile_pool(name="tp8a", bufs=2))
        tp8b = ctx.enter_context(tc.tile_pool(name="tp8b", bufs=2))
        tp8c = ctx.enter_context(tc.tile_pool(name="tp8c", bufs=2))
        tp4 = ctx.enter_context(tc.tile_pool(name="tp4", bufs=2))
        tp2 = ctx.enter_context(tc.tile_pool(name="tp2", bufs=2))
        tp1 = ctx.enter_context(tc.tile_pool(name="tp1", bufs=2))
        sbpool = ctx.enter_context(tc.tile_pool(name="sbpool", bufs=3))
        invbpool = ctx.enter_context(tc.tile_pool(name="invbpool", bufs=3))
        xspool = ctx.enter_context(tc.tile_pool(name="xspool", bufs=M_TILES))
        wsbpool = ctx.enter_context(tc.tile_pool(name="wsbpool", bufs=4))
        biaspool = ctx.enter_context(tc.tile_pool(name="biaspool", bufs=4))
        opool = ctx.enter_context(tc.tile_pool(name="opool", bufs=6))
        psum_mm = ctx.enter_context(tc.tile_pool(name="psum_mm", bufs=8, space="PSUM"))

        wslab_tiles = [None] * N_BLKS
        wsb_tiles = [None] * N_BLKS
        bias_tiles = [None] * N_BLKS
        xs_tiles = [None] * M_TILES
        xq_half = [[None, None] for _ in range(M_TILES)]

        def issue_wslab(nb, split):
            t = wpool.tile([P, K_SUBS, N_BLK], FP8, tag="w")
            if split:
                nc.scalar.dma_start(out=t[:, 0:KH, :], in_=wt_ap[nb, :, 0:KH, :])
                nc.sync.dma_start(out=t[:, KH:, :], in_=wt_ap[nb, :, KH:, :])
            else:
                nc.scalar.dma_start(out=t[:], in_=wt_ap[nb])
            wslab_tiles[nb] = t

        def issue_wsb(nb):
            w = wsbpool.tile([P, N_BLK], F32, tag="wsb")
            nc.scalar.dma_start(
                out=w[:],
                in_=bass.AP(tensor=ws_d[:].tensor, offset=nb * N_BLK,
                            ap=[[0, P], [1, N_BLK]]),
            )
            wsb_tiles[nb] = w
            b = biaspool.tile([P, N_BLK], BF16, tag="biasb")
            nc.scalar.dma_start(
                out=b[:],
                in_=bass.AP(tensor=bias_d[:].tensor, offset=nb * N_BLK,
                            ap=[[0, P], [1, N_BLK]]),
            )
            bias_tiles[nb] = b

        xT_tiles = [None] * M_TILES

        def issue_transposes(mt):
            # XBAR transpose pieces straight from DRAM (SP ring only)
            xTs = []
            for h in range(2):
                xT = (xTpoolA if h == 0 else xTpoolB).tile([P, KH, P], BF16, tag=f"xT{h}")
                nc.sync.dma_start_transpose(
                    out=xT[:],
                    in_=x_ap[mt * P:(mt + 1) * P, h * (K // 2):(h + 1) * (K // 2)])
                xTs.append(xT)
            xT_tiles[mt] = xTs

        def issue_chain(mt):
            xTa, xTb = xTs = xT_tiles[mt]

            # amax: ACT |x| (exact sign-clear), then DVE max tree
            abA = tpabsA.tile([P, KH, P], BF16, tag="abA")
            nc.scalar.activation(out=abA[:], in_=xTa[:], func=mybir.ActivationFunctionType.Abs)
            abB = tpabsB.tile([P, KH, P], BF16, tag="abB")
            nc.scalar.activation(out=abB[:], in_=xTb[:], func=mybir.ActivationFunctionType.Abs)
            m1 = tp8a.tile([P, 8, P], BF16, tag="m1")
            nc.vector.tensor_tensor(out=m1[:], in0=abA[:, 0:8, :], in1=abA[:, 8:16, :],
                                    op=mybir.AluOpType.max)
            m2 = tp8b.tile([P, 8, P], BF16, tag="m2")
            nc.vector.tensor_tensor(out=m2[:], in0=abB[:, 0:8, :], in1=abB[:, 8:16, :],
                                    op=mybir.AluOpType.max)
            m3 = tp8c.tile([P, 8, P], BF16, tag="m3")
            nc.vector.tensor_tensor(out=m3[:], in0=m1[:], in1=m2[:],
                                    op=mybir.AluOpType.max)
            m4 = tp4.tile([P, 4, P], BF16, tag="m4")
            nc.vector.tensor_tensor(out=m4[:], in0=m3[:, 0:4, :], in1=m3[:, 4:8, :],
                                    op=mybir.AluOpType.max)
            m5 = tp2.tile([P, 2, P], BF16, tag="m5")
            nc.vector.tensor_tensor(out=m5[:], in0=m4[:, 0:2, :], in1=m4[:, 2:4, :],
                                    op=mybir.AluOpType.max)
            m6 = tp1.tile([P, P], BF16, tag="m6")
            nc.vector.tensor_tensor(out=m6[:], in0=m5[:, 0, :], in1=m5[:, 1, :],
                                    op=mybir.AluOpType.max)

            # all-reduce across partitions -> amax[m] broadcast to every row
            allr = tp1.tile([P, P], F32, tag="allr")
            nc.gpsimd.partition_all_reduce(allr[:], m6[:], channels=P,
                                           reduce_op=bass_isa.ReduceOp.max)

            with tc.high_priority():
                xsb = sbpool.tile([P, P], F32, tag="xsb")
                nc.vector.tensor_scalar(
                    out=xsb[:], in0=allr[:],
                    scalar1=1e-10, scalar2=1.0 / 224.0,
                    op0=mybir.AluOpType.max, op1=mybir.AluOpType.mult,
                )
                invb = invbpool.tile([P, P], F32, tag="invb")
                nc.vector.reciprocal(out=invb[:], in_=xsb[:])

            # xs (partition-major, for the output stage) via tiny DRAM bounce
            st1 = nc.sync.dma_start(out=xs_scr[mt], in_=xsb[0:1, :])
            xs = xspool.tile([P, 1], F32, tag="xs")
            st2 = nc.sync.dma_start(
                out=xs[:],
                in_=bass.AP(tensor=xs_scr[:].tensor, offset=mt * P,
                            ap=[[1, P], [1, 1]]),
            )
            tile.add_dep_helper(st2.ins, st1.ins, sync=True, reason="xs scratch raw")
            xs_tiles[mt] = xs

            # quantize pieces: DVE for A, GpSimd for B (fp8 out)
            ib = invb[:]
            in1 = bass.AP(tensor=ib.tensor, offset=ib.offset,
                          ap=[ib.ap[0], [0, KH], ib.ap[1]])
            for h, xT in enumerate(xTs):
                xq = (xqpoolA if h == 0 else xqpoolB).tile([P, KH, P], FP8, tag=f"xq{h}")
                eng = nc.vector if h == 0 else nc.gpsimd
                eng.tensor_tensor(out=xq[:], in0=xT[:], in1=in1,
                                  op=mybir.AluOpType.mult)
                xq_half[mt][h] = xq

        # ---- preamble ----
        # scalar ring: w0 first (gates MM0), then small broadcasts, then the
        # scalar half of w1.  sync ring: all 16 XBAR transposes up front
        # (paced by the xT pool WAR deps, which drain at quantize speed),
        # with w1's sync half woven in after mt2's pieces.
        issue_wslab(0, split=False)
        issue_wsb(0)
        for mt in range(3):
            issue_transposes(mt)
        issue_wslab(1, split=True)
        for mt in range(3, M_TILES):
            issue_transposes(mt)
        issue_wsb(1)
        issue_chain(0)
        issue_chain(1)
        issue_chain(2)

        # ---- main GEMM: rectangle-grow order over (nb, mt) ----
        order = _gemm_order()
        pre_slot = {0: [("c", 3)], 2: [("c", 4)], 4: [("c", 5)],
                    6: [("c", 6)], 8: [("c", 7), ("w", 2), ("b", 2)],
                    16: [("w", 3), ("b", 3)], 24: [("w", 4), ("b", 4)],
                    33: [("w", 5), ("b", 5)], 41: [("w", 6), ("b", 6)],
                    49: [("w", 7), ("b", 7)]}
        wsb_done = {0, 1}
        for s, (nb, mt) in enumerate(order):
            for kind, idx in pre_slot.get(s, []):
                if kind == "c":
                    issue_chain(idx)
                elif kind == "w":
                    issue_wslab(idx, split=False)
                else:
                    issue_wsb(idx)
            wslab = wslab_tiles[nb]
            pm = psum_mm.tile([P, N_BLK], F32, tag="pm")
            for j in range(K_SUPERS):
                h, jj = divmod(j, 8)
                nc.tensor.matmul(
                    out=pm[:],
                    lhsT=xq_half[mt][h][:, 2 * jj:2 * jj + 2, :],
                    rhs=wslab[:, 2 * j:2 * j + 2, :],
                    start=(j == 0), stop=(j == K_SUPERS - 1),
                    perf_mode=mybir.MatmulPerfMode.DoubleRow,
                )
            sb = opool.tile([P, N_BLK], BF16, tag="sb")
            nc.vector.scalar_tensor_tensor(
                out=sb[:], in0=pm[:], scalar=xs_tiles[mt][:],
                in1=wsb_tiles[nb][:],
                op0=mybir.AluOpType.mult, op1=mybir.AluOpType.mult,
            )
            nc.gpsimd.tensor_tensor(out=sb[:], in0=sb[:],
                                    in1=bias_tiles[nb][:],
                                    op=mybir.AluOpType.add)
            nc.sync.dma_start(
                out=out_ap[mt * P:(mt + 1) * P, nb * N_BLK:(nb + 1) * N_BLK],
                in_=sb[:],
            )

    nc.compile()
    return nc


def _get_program():
    if "nc" not in _PROGRAM_CACHE:
        _PROGRAM_CACHE["nc"] = _build_program()
    return _PROGRAM_CACHE["nc"]


def _run_sharded(x, weight, weight_scales, bias, trace=False):
    x = np.asarray(x).astype(ml_dtypes.bfloat16, copy=False)
    weight = np.asarray(weight, dtype=np.float32)
    weight_scales = np.asarray(weight_scales, dtype=np.float32)
    bias16 = np.asarray(bias, dtype=np.float32).astype(ml_dtypes.bfloat16)

    # host-side sharding / layout only (lossless fp8 re-encode of weights)
    wt = np.ascontiguousarray(
        weight.T.reshape(K_SUBS, P, N_BLKS, N_BLK).transpose(2, 1, 0, 3)
    ).astype(ml_dtypes.float8_e4m3)
    in_maps = []
    for c in range(NCORES):
        in_maps.append({
            "x": np.ascontiguousarray(x[c * M_SHARD:(c + 1) * M_SHARD]),
            "wt": wt,
            "ws": weight_scales,
            "bias": bias16,
        })

    nc = _get_program()
    res = run_bass_kernel_spmd(nc, in_maps, core_ids=list(range(NCORES)), trace=trace)
    out = np.concatenate([res.results[c]["out"] for c in range(NCORES)], axis=0)
    return out, res.exec_time_ns


def kernel(x, weight, weight_scales, bias):
    out, _ = _run_sharded(x, weight, weight_scales, bias,
                          trace=bool(os.environ.get("KERNEL_TRACE")))
    return out


# revision 14
# speedup vs baseline: 1.1347x; 1.1347x over previous
"""Fp8 per-token/per-channel quantized linear for Trainium2, 8 NeuronCores.

Computation (matches the jax reference):
    amax[m]  = max_k |x[m, k]|                       (x is bf16)
    xs[m]    = max(amax, 1e-10) / 448
    x_q      = e4m3fn_round(x / xs)                  (values up to +-448)
    out      = bf16((x_q @ W^T) * xs * w_scales) + bf16(bias)

Mapping to TRN2 hardware:
  * TRN's fp8 E4M3 saturates at +-240, so we quantize at HALF scale
    (factor folded into the output scale; exact on fp8's power-of-2 grid).
  * Sharding: row-parallel over M (8 cores x 1024 rows of x each); the full
    fp8-re-encoded weight streams through every core.
  * x is read from DRAM exactly once, via the DMA XBAR transpose
    (dma_start_transpose, SP ring only -- it corrupts data on the ACT ring)
    straight into the [k_lo, k_sub, m] layout the DoubleRow GEMM wants.
    There is no row-major x load at all: amax comes from the transposed
    pieces via a DVE abs_max tree + GpSimd partition_all_reduce(max), which
    also yields the quant scale already partition-broadcast (invb) with no
    DRAM round trip.  Only the output scale xs needs a tiny [1,128] ->
    DRAM -> [128,1] bounce.
  * Quantize: DVE multiplies piece A (16 ksubs), GpSimd piece B, fp8 out.
  * Ring budget (~125 GB/s per ring, 2 HWDGE rings): SP carries transposes
    (~8MB effective) + outputs (8MB) + scale bounces; ACT carries weights
    (16MB) + ws/bias block broadcasts.  Early weight slabs w1-w3 are split
    across both rings.  The GEMM walks (nb, mt) in rectangle-grow order so
    the early phase needs only a small corner of x-tiles x w-slabs.
  * Output stage fused: DVE scalar_tensor_tensor (psum*xs)*ws -> bf16,
    GpSimd adds host-precast bf16 bias, SP-ring DMA out.
  * PE runs ONLY the 1024 fp8 DoubleRow matmuls (k=256, n=512 each) with
    all 8 PSUM banks in flight.
"""

import os
import numpy as np
import ml_dtypes
from contextlib import ExitStack

import concourse.bass as bass
import concourse.bacc as bacc
import concourse.tile as tile
from concourse import mybir, bass_isa
from concourse.bass_utils import run_bass_kernel_spmd

P = 128
M, K, N = 8192, 4096, 4096
NCORES = 8
M_SHARD = M // NCORES          # 1024 rows of x per core
M_TILES = M_SHARD // P         # 8
K_SUBS = K // P                # 32
KH = K_SUBS // 2               # 16 ksubs per transpose/quantize piece
K_SUPERS = K // (2 * P)        # 16 (DoubleRow consumes 256 rows of K)
N_BLK = 512
N_BLKS = N // N_BLK            # 8

FP8 = mybir.dt.float8e4
F32 = mybir.dt.float32
BF16 = mybir.dt.bfloat16

_PROGRAM_CACHE = {}


def _gemm_order():
    """Rectangle-grow (staircase) enumeration of (nb, mt), mt-biased 4:1."""
    order = [(0, 0)]
    nm, nn = 1, 1
    while nm < M_TILES or nn < N_BLKS:
        if nm < M_TILES and (nm < 4 * nn or nn == N_BLKS):
            order.extend((nb, nm) for nb in range(nn))
            nm += 1
        else:
            order.extend((nn, mt) for mt in range(nm))
            nn += 1
    return order


def _build_program():
    nc = bacc.Bacc(None, target_bir_lowering=False)

    x_d = nc.declare_dram_parameter("x", [M_SHARD, K], BF16, isOutput=False)
    # host layout: wt[nb, p, ksub, n] = weight[nb*512 + n, ksub*128 + p], fp8
    wt_d = nc.declare_dram_parameter("wt", [N_BLKS, P, K_SUBS, N_BLK], FP8, isOutput=False)
    ws_d = nc.declare_dram_parameter("ws", [N], F32, isOutput=False)
    bias_d = nc.declare_dram_parameter("bias", [N], BF16, isOutput=False)
    out_d = nc.declare_dram_parameter("out", [M_SHARD, N], BF16, isOutput=True)

    xs_scr = nc.dram_tensor("xs_scratch", [M_TILES, P], F32, kind="Internal")

    x_ap = x_d[:]
    wt_ap = wt_d[:]
    out_ap = out_d[:]

    with tile.TileContext(nc) as tc, ExitStack() as ctx:
        xTpoolA = ctx.enter_context(tc.tile_pool(name="xTpoolA", bufs=6))
        xTpoolB = ctx.enter_context(tc.tile_pool(name="xTpoolB", bufs=6))
        xqpoolA = ctx.enter_context(tc.tile_pool(name="xqpoolA", bufs=M_TILES))
        xqpoolB = ctx.enter_context(tc.tile_pool(name="xqpoolB", bufs=M_TILES))
        wpool = ctx.enter_context(tc.tile_pool(name="wpool", bufs=4))
        tpabsA = ctx.enter_context(tc.tile_pool(name="tpabsA", bufs=2))
        tpabsB = ctx.enter_context(tc.tile_pool(name="tpabsB", bufs=2))
        tp8a = ctx.enter_context(tc.tile_pool(name="tp8a", bufs=2))
        tp8b = ctx.enter_context(tc.tile_pool(name="tp8b", bufs=2))
        tp8c = ctx.enter_context(tc.tile_pool(name="tp8c", bufs=2))
        tp4 = ctx.enter_context(tc.tile_pool(name="tp4", bufs=2))
        tp2 = ctx.enter_context(tc.tile_pool(name="tp2", bufs=2))
        tp1 = ctx.enter_context(tc.tile_pool(name="tp1", bufs=2))
        sbpool = ctx.enter_context(tc.tile_pool(name="sbpool", bufs=3))
        invbpool = ctx.enter_context(tc.tile_pool(name="invbpool", bufs=3))
        xspool = ctx.enter_context(tc.tile_pool(name="xspool", bufs=M_TILES))
        wsbpool = ctx.enter_context(tc.tile_pool(name="wsbpool", bufs=4))
        biaspool = ctx.enter_context(tc.tile_pool(name="biaspool", bufs=4))
        opool = ctx.enter_context(tc.tile_pool(name="opool", bufs=6))
        psum_mm = ctx.enter_context(tc.tile_pool(name="psum_mm", bufs=8, space="PSUM"))

        wslab_tiles = [None] * N_BLKS
        wsb_tiles = [None] * N_BLKS
        bias_tiles = [None] * N_BLKS
        xs_tiles = [None] * M_TILES
        xq_half = [[None, None] for _ in range(M_TILES)]

        def issue_wslab(nb, split):
            t = wpool.tile([P, K_SUBS, N_BLK], FP8, tag="w")
            if split:
                nc.scalar.dma_start(out=t[:, 0:KH, :], in_=wt_ap[nb, :, 0:KH, :])
                nc.sync.dma_start(out=t[:, KH:, :], in_=wt_ap[nb, :, KH:, :])
            else:
                nc.scalar.dma_start(out=t[:], in_=wt_ap[nb])
            wslab_tiles[nb] = t

        def issue_wsb(nb):
            w = wsbpool.tile([P, N_BLK], F32, tag="wsb")
            nc.scalar.dma_start(
                out=w[:],
                in_=bass.AP(tensor=ws_d[:].tensor, offset=nb * N_BLK,
                            ap=[[0, P], [1, N_BLK]]),
            )
            wsb_tiles[nb] = w
            b = biaspool.tile([P, N_BLK], BF16, tag="biasb")
            nc.scalar.dma_start(
                out=b[:],
                in_=bass.AP(tensor=bias_d[:].tensor, offset=nb * N_BLK,
                            ap=[[0, P], [1, N_BLK]]),
            )
            bias_tiles[nb] = b

        xT_tiles = [None] * M_TILES

        def issue_transposes(mt):
            # XBAR transpose pieces straight from DRAM (SP ring only)
            xTs = []
            for h in range(2):
                xT = (xTpoolA if h == 0 else xTpoolB).tile([P, KH, P], BF16, tag=f"xT{h}")
                nc.sync.dma_start_transpose(
                    out=xT[:],
                    in_=x_ap[mt * P:(mt + 1) * P, h * (K // 2):(h + 1) * (K // 2)])
                xTs.append(xT)
            xT_tiles[mt] = xTs

        def issue_chain(mt):
            xTa, xTb = xTs = xT_tiles[mt]

            # amax: ACT |x| (exact sign-clear), then DVE max tree
            abA = tpabsA.tile([P, KH, P], BF16, tag="abA")
            nc.scalar.activation(out=abA[:], in_=xTa[:], func=mybir.ActivationFunctionType.Abs)
            abB = tpabsB.tile([P, KH, P], BF16, tag="abB")
            nc.scalar.activation(out=abB[:], in_=xTb[:], func=mybir.ActivationFunctionType.Abs)
            m1 = tp8a.tile([P, 8, P], BF16, tag="m1")
            nc.vector.tensor_tensor(out=m1[:], in0=abA[:, 0:8, :], in1=abA[:, 8:16, :],
                                    op=mybir.AluOpType.max)
            m2 = tp8b.tile([P, 8, P], BF16, tag="m2")
            nc.vector.tensor_tensor(out=m2[:], in0=abB[:, 0:8, :], in1=abB[:, 8:16, :],
                                    op=mybir.AluOpType.max)
            m3 = tp8c.tile([P, 8, P], BF16, tag="m3")
            nc.vector.tensor_tensor(out=m3[:], in0=m1[:], in1=m2[:],
                                    op=mybir.AluOpType.max)
            m4 = tp4.tile([P, 4, P], BF16, tag="m4")
            nc.vector.tensor_tensor(out=m4[:], in0=m3[:, 0:4, :], in1=m3[:, 4:8, :],
                                    op=mybir.AluOpType.max)
            m5 = tp2.tile([P, 2, P], BF16, tag="m5")
            nc.vector.tensor_tensor(out=m5[:], in0=m4[:, 0:2, :], in1=m4[:, 2:4, :],
                                    op=mybir.AluOpType.max)
            m6 = tp1.tile([P, P], BF16, tag="m6")
            nc.vector.tensor_tensor(out=m6[:], in0=m5[:, 0, :], in1=m5[:, 1, :],
                                    op=mybir.AluOpType.max)

            # all-reduce across partitions -> amax[m] broadcast to every row
            allr = tp1.tile([P, P], F32, tag="allr")
            nc.gpsimd.partition_all_reduce(allr[:], m6[:], channels=P,
                                           reduce_op=bass_isa.ReduceOp.max)

            with tc.high_priority():
                xsb = sbpool.tile([P, P], F32, tag="xsb")
                nc.vector.tensor_scalar(
                    out=xsb[:], in0=allr[:],
                    scalar1=1e-10, scalar2=1.0 / 224.0,
                    op0=mybir.AluOpType.max, op1=mybir.AluOpType.mult,
                )
                invb = invbpool.tile([P, P], F32, tag="invb")
                nc.vector.reciprocal(out=invb[:], in_=xsb[:])

            # xs (partition-major, for the output stage) via tiny DRAM bounce
            st1 = nc.sync.dma_start(out=xs_scr[mt], in_=xsb[0:1, :])
            xs = xspool.tile([P, 1], F32, tag="xs")
            st2 = nc.sync.dma_start(
                out=xs[:],
                in_=bass.AP(tensor=xs_scr[:].tensor, offset=mt * P,
                            ap=[[1, P], [1, 1]]),
            )
            tile.add_dep_helper(st2.ins, st1.ins, sync=True, reason="xs scratch raw")
            xs_tiles[mt] = xs

            # quantize pieces: DVE for A, GpSimd for B (fp8 out)
            ib = invb[:]
            in1 = bass.AP(tensor=ib.tensor, offset=ib.offset,
                          ap=[ib.ap[0], [0, KH], ib.ap[1]])
            for h, xT in enumerate(xTs):
                xq = (xqpoolA if h == 0 else xqpoolB).tile([P, KH, P], FP8, tag=f"xq{h}")
                eng = nc.vector if h == 0 else nc.gpsimd
                eng.tensor_tensor(out=xq[:], in0=xT[:], in1=in1,
                                  op=mybir.AluOpType.mult)
                xq_half[mt][h] = xq

        # ---- preamble ----
        # The scheduler serializes every DMA-transpose against the most
        # recently issued regular DMA (and vice versa), so transposes go
        # FIRST -- before w0 -- or tr0 stalls behind w0's 16us transfer.
        # All 16 XBAR transposes are issued up front (paced by the deep xT
        # pool WAR deps at quantize speed); weight slabs stay whole on the
        # scalar ring, woven between transpose groups.
        issue_transposes(0)
        issue_wslab(0, split=False)
        issue_transposes(1)
        issue_transposes(2)
        issue_wslab(1, split=False)
        for mt in range(3, M_TILES):
            issue_transposes(mt)
        issue_wsb(0)
        issue_wsb(1)
        issue_chain(0)
        issue_chain(1)
        issue_chain(2)

        # ---- main GEMM: rectangle-grow order over (nb, mt) ----
        order = _gemm_order()
        pre_slot = {0: [("c", 3)], 2: [("c", 4)], 4: [("c", 5), ("w", 2)],
                    6: [("c", 6)], 8: [("c", 7), ("b", 2)],
                    14: [("w", 3)], 16: [("b", 3)], 24: [("w", 4), ("b", 4)],
                    33: [("w", 5), ("b", 5)], 41: [("w", 6), ("b", 6)],
                    49: [("w", 7), ("b", 7)]}
        wsb_done = {0, 1}
        for s, (nb, mt) in enumerate(order):
            for kind, idx in pre_slot.get(s, []):
                if kind == "c":
                    issue_chain(idx)
                elif kind == "w":
                    issue_wslab(idx, split=False)
                else:
                    issue_wsb(idx)
            wslab = wslab_tiles[nb]
            pm = psum_mm.tile([P, N_BLK], F32, tag="pm")
            for j in range(K_SUPERS):
                h, jj = divmod(j, 8)
                nc.tensor.matmul(
                    out=pm[:],
                    lhsT=xq_half[mt][h][:, 2 * jj:2 * jj + 2, :],
                    rhs=wslab[:, 2 * j:2 * j + 2, :],
                    start=(j == 0), stop=(j == K_SUPERS - 1),
                    perf_mode=mybir.MatmulPerfMode.DoubleRow,
                )
            sb = opool.tile([P, N_BLK], BF16, tag="sb")
            nc.vector.scalar_tensor_tensor(
                out=sb[:], in0=pm[:], scalar=xs_tiles[mt][:],
                in1=wsb_tiles[nb][:],
                op0=mybir.AluOpType.mult, op1=mybir.AluOpType.mult,
            )
            nc.gpsimd.tensor_tensor(out=sb[:], in0=sb[:],
                                    in1=bias_tiles[nb][:],
                                    op=mybir.AluOpType.add)
            nc.sync.dma_start(
                out=out_ap[mt * P:(mt + 1) * P, nb * N_BLK:(nb + 1) * N_BLK],
                in_=sb[:],
            )

    nc.compile()
    return nc


def _get_program():
    if "nc" not in _PROGRAM_CACHE:
        _PROGRAM_CACHE["nc"] = _build_program()
    return _PROGRAM_CACHE["nc"]


def _run_sharded(x, weight, weight_scales, bias, trace=False):
    x = np.asarray(x).astype(ml_dtypes.bfloat16, copy=False)
    weight = np.asarray(weight, dtype=np.float32)
    weight_scales = np.asarray(weight_scales, dtype=np.float32)
    bias16 = np.asarray(bias, dtype=np.float32).astype(ml_dtypes.bfloat16)

    # host-side sharding / layout only (lossless fp8 re-encode of weights)
    wt = np.ascontiguousarray(
        weight.T.reshape(K_SUBS, P, N_BLKS, N_BLK).transpose(2, 1, 0, 3)
    ).astype(ml_dtypes.float8_e4m3)
    in_maps = []
    for c in range(NCORES):
        in_maps.append({
            "x": np.ascontiguousarray(x[c * M_SHARD:(c + 1) * M_SHARD]),
            "wt": wt,
            "ws": weight_scales,
            "bias": bias16,
        })

    nc = _get_program()
    res = run_bass_kernel_spmd(nc, in_maps, core_ids=list(range(NCORES)), trace=trace)
    out = np.concatenate([res.results[c]["out"] for c in range(NCORES)], axis=0)
    return out, res.exec_time_ns


def kernel(x, weight, weight_scales, bias):
    out, _ = _run_sharded(x, weight, weight_scales, bias,
                          trace=bool(os.environ.get("KERNEL_TRACE")))
    return out


# revision 15
# speedup vs baseline: 1.1617x; 1.0239x over previous
"""Fp8 per-token/per-channel quantized linear for Trainium2, 8 NeuronCores.

Computation (matches the jax reference):
    amax[m]  = max_k |x[m, k]|                       (x is bf16)
    xs[m]    = max(amax, 1e-10) / 448
    x_q      = e4m3fn_round(x / xs)                  (values up to +-448)
    out      = bf16((x_q @ W^T) * xs * w_scales) + bf16(bias)

Mapping to TRN2 hardware:
  * TRN's fp8 E4M3 saturates at +-240, so we quantize at HALF scale
    (factor folded into the output scale; exact on fp8's power-of-2 grid).
  * Sharding: row-parallel over M (8 cores x 1024 rows of x each); the full
    fp8-re-encoded weight streams through every core.
  * x is read from DRAM exactly once, via the DMA XBAR transpose
    (dma_start_transpose, SP ring only -- it corrupts data on the ACT ring)
    straight into the [k_lo, k_sub, m] layout the DoubleRow GEMM wants.
    There is no row-major x load at all: amax comes from the transposed
    pieces via a DVE abs_max tree + GpSimd partition_all_reduce(max), which
    also yields the quant scale already partition-broadcast (invb) with no
    DRAM round trip.  Only the output scale xs needs a tiny [1,128] ->
    DRAM -> [128,1] bounce.
  * Quantize: DVE multiplies piece A (16 ksubs), GpSimd piece B, fp8 out.
  * Ring budget (~125 GB/s per ring, 2 HWDGE rings): SP carries transposes
    (~8MB effective) + outputs (8MB) + scale bounces; ACT carries weights
    (16MB) + ws/bias block broadcasts.  Early weight slabs w1-w3 are split
    across both rings.  The GEMM walks (nb, mt) in rectangle-grow order so
    the early phase needs only a small corner of x-tiles x w-slabs.
  * Output stage fused: DVE scalar_tensor_tensor (psum*xs)*ws -> bf16,
    GpSimd adds host-precast bf16 bias, SP-ring DMA out.
  * PE runs ONLY the 1024 fp8 DoubleRow matmuls (k=256, n=512 each) with
    all 8 PSUM banks in flight.
"""

import os
import numpy as np
import ml_dtypes
from contextlib import ExitStack

import concourse.bass as bass
import concourse.bacc as bacc
import concourse.tile as tile
from concourse import mybir, bass_isa
from concourse.bass_utils import run_bass_kernel_spmd

P = 128
M, K, N = 8192, 4096, 4096
NCORES = 8
M_SHARD = M // NCORES          # 1024 rows of x per core
M_TILES = M_SHARD // P         # 8
K_SUBS = K // P                # 32
KH = K_SUBS // 2               # 16 ksubs per transpose/quantize piece
K_SUPERS = K // (2 * P)        # 16 (DoubleRow consumes 256 rows of K)
N_BLK = 512
N_BLKS = N // N_BLK            # 8

FP8 = mybir.dt.float8e4
F32 = mybir.dt.float32
BF16 = mybir.dt.bfloat16

_PROGRAM_CACHE = {}


def _gemm_order():
    """Rectangle-grow (staircase) enumeration of (nb, mt), mt-biased 4:1."""
    order = [(0, 0)]
    nm, nn = 1, 1
    while nm < M_TILES or nn < N_BLKS:
        if nm < M_TILES and (nm < 4 * nn or nn == N_BLKS):
            order.extend((nb, nm) for nb in range(nn))
            nm += 1
        else:
            order.extend((nn, mt) for mt in range(nm))
            nn += 1
    return order


def _build_program():
    nc = bacc.Bacc(None, target_bir_lowering=False)

    x_d = nc.declare_dram_parameter("x", [M_SHARD, K], BF16, isOutput=False)
    # host layout: wt[nb, p, ksub, n] = weight[nb*512 + n, ksub*128 + p], fp8
    wt_d = nc.declare_dram_parameter("wt", [N_BLKS, P, K_SUBS, N_BLK], FP8, isOutput=False)
    ws_d = nc.declare_dram_parameter("ws", [N], F32, isOutput=False)
    bias_d = nc.declare_dram_parameter("bias", [N], BF16, isOutput=False)
    out_d = nc.declare_dram_parameter("out", [M_SHARD, N], BF16, isOutput=True)

    xs_scr = nc.dram_tensor("xs_scratch", [M_TILES, P], F32, kind="Internal")

    x_ap = x_d[:]
    wt_ap = wt_d[:]
    out_ap = out_d[:]

    with tile.TileContext(nc) as tc, ExitStack() as ctx:
        xTpoolA = ctx.enter_context(tc.tile_pool(name="xTpoolA", bufs=6))
        xTpoolB = ctx.enter_context(tc.tile_pool(name="xTpoolB", bufs=6))
        xqpoolA = ctx.enter_context(tc.tile_pool(name="xqpoolA", bufs=M_TILES))
        xqpoolB = ctx.enter_context(tc.tile_pool(name="xqpoolB", bufs=M_TILES))
        wpool = ctx.enter_context(tc.tile_pool(name="wpool", bufs=4))
        tpabsA = ctx.enter_context(tc.tile_pool(name="tpabsA", bufs=2))
        tpabsB = ctx.enter_context(tc.tile_pool(name="tpabsB", bufs=2))
        tp8a = ctx.enter_context(tc.tile_pool(name="tp8a", bufs=2))
        tp8b = ctx.enter_context(tc.tile_pool(name="tp8b", bufs=2))
        tp8c = ctx.enter_context(tc.tile_pool(name="tp8c", bufs=2))
        tp4 = ctx.enter_context(tc.tile_pool(name="tp4", bufs=2))
        tp2 = ctx.enter_context(tc.tile_pool(name="tp2", bufs=2))
        tp1 = ctx.enter_context(tc.tile_pool(name="tp1", bufs=2))
        sbpool = ctx.enter_context(tc.tile_pool(name="sbpool", bufs=3))
        invbpool = ctx.enter_context(tc.tile_pool(name="invbpool", bufs=3))
        xspool = ctx.enter_context(tc.tile_pool(name="xspool", bufs=M_TILES))
        wsbpool = ctx.enter_context(tc.tile_pool(name="wsbpool", bufs=4))
        biaspool = ctx.enter_context(tc.tile_pool(name="biaspool", bufs=4))
        opool = ctx.enter_context(tc.tile_pool(name="opool", bufs=8))
        psum_mm = ctx.enter_context(tc.tile_pool(name="psum_mm", bufs=8, space="PSUM"))

        wslab_tiles = [None] * N_BLKS
        wsb_tiles = [None] * N_BLKS
        bias_tiles = [None] * N_BLKS
        xs_tiles = [None] * M_TILES
        xq_half = [[None, None] for _ in range(M_TILES)]

        def issue_wslab(nb, split):
            t = wpool.tile([P, K_SUBS, N_BLK], FP8, tag="w")
            if split:
                nc.scalar.dma_start(out=t[:, 0:KH, :], in_=wt_ap[nb, :, 0:KH, :])
                nc.sync.dma_start(out=t[:, KH:, :], in_=wt_ap[nb, :, KH:, :])
            else:
                nc.scalar.dma_start(out=t[:], in_=wt_ap[nb])
            wslab_tiles[nb] = t

        def issue_wsb(nb):
            w = wsbpool.tile([P, N_BLK], F32, tag="wsb")
            nc.scalar.dma_start(
                out=w[:],
                in_=bass.AP(tensor=ws_d[:].tensor, offset=nb * N_BLK,
                            ap=[[0, P], [1, N_BLK]]),
            )
            wsb_tiles[nb] = w
            b = biaspool.tile([P, N_BLK], BF16, tag="biasb")
            nc.scalar.dma_start(
                out=b[:],
                in_=bass.AP(tensor=bias_d[:].tensor, offset=nb * N_BLK,
                            ap=[[0, P], [1, N_BLK]]),
            )
            bias_tiles[nb] = b

        xT_tiles = [None] * M_TILES

        def issue_transposes(mt):
            # XBAR transpose pieces straight from DRAM (SP ring only)
            xTs = []
            for h in range(2):
                xT = (xTpoolA if h == 0 else xTpoolB).tile([P, KH, P], BF16, tag=f"xT{h}")
                nc.sync.dma_start_transpose(
                    out=xT[:],
                    in_=x_ap[mt * P:(mt + 1) * P, h * (K // 2):(h + 1) * (K // 2)])
                xTs.append(xT)
            xT_tiles[mt] = xTs

        def issue_chain(mt):
            xTa, xTb = xTs = xT_tiles[mt]

            # amax: ACT |x| (exact sign-clear), then DVE max tree
            abA = tpabsA.tile([P, KH, P], BF16, tag="abA")
            nc.scalar.activation(out=abA[:], in_=xTa[:], func=mybir.ActivationFunctionType.Abs)
            abB = tpabsB.tile([P, KH, P], BF16, tag="abB")
            nc.scalar.activation(out=abB[:], in_=xTb[:], func=mybir.ActivationFunctionType.Abs)
            m1 = tp8a.tile([P, 8, P], BF16, tag="m1")
            nc.vector.tensor_tensor(out=m1[:], in0=abA[:, 0:8, :], in1=abA[:, 8:16, :],
                                    op=mybir.AluOpType.max)
            m2 = tp8b.tile([P, 8, P], BF16, tag="m2")
            nc.vector.tensor_tensor(out=m2[:], in0=abB[:, 0:8, :], in1=abB[:, 8:16, :],
                                    op=mybir.AluOpType.max)
            m3 = tp8c.tile([P, 8, P], BF16, tag="m3")
            nc.vector.tensor_tensor(out=m3[:], in0=m1[:], in1=m2[:],
                                    op=mybir.AluOpType.max)
            m4 = tp4.tile([P, 4, P], BF16, tag="m4")
            nc.vector.tensor_tensor(out=m4[:], in0=m3[:, 0:4, :], in1=m3[:, 4:8, :],
                                    op=mybir.AluOpType.max)
            m5 = tp2.tile([P, 2, P], BF16, tag="m5")
            nc.vector.tensor_tensor(out=m5[:], in0=m4[:, 0:2, :], in1=m4[:, 2:4, :],
                                    op=mybir.AluOpType.max)
            m6 = tp1.tile([P, P], BF16, tag="m6")
            nc.vector.tensor_tensor(out=m6[:], in0=m5[:, 0, :], in1=m5[:, 1, :],
                                    op=mybir.AluOpType.max)

            # all-reduce across partitions -> amax[m] broadcast to every row
            allr = tp1.tile([P, P], F32, tag="allr")
            nc.gpsimd.partition_all_reduce(allr[:], m6[:], channels=P,
                                           reduce_op=bass_isa.ReduceOp.max)

            with tc.high_priority():
                xsb = sbpool.tile([P, P], F32, tag="xsb")
                nc.vector.tensor_scalar(
                    out=xsb[:], in0=allr[:],
                    scalar1=1e-10, scalar2=1.0 / 224.0,
                    op0=mybir.AluOpType.max, op1=mybir.AluOpType.mult,
                )
                invb = invbpool.tile([P, P], F32, tag="invb")
                nc.vector.reciprocal(out=invb[:], in_=xsb[:])

            # xs (partition-major, for the output stage) via tiny DRAM bounce
            st1 = nc.sync.dma_start(out=xs_scr[mt], in_=xsb[0:1, :])
            xs = xspool.tile([P, 1], F32, tag="xs")
            st2 = nc.sync.dma_start(
                out=xs[:],
                in_=bass.AP(tensor=xs_scr[:].tensor, offset=mt * P,
                            ap=[[1, P], [1, 1]]),
            )
            tile.add_dep_helper(st2.ins, st1.ins, sync=True, reason="xs scratch raw")
            xs_tiles[mt] = xs

            # quantize pieces: DVE for A, GpSimd for B (fp8 out)
            ib = invb[:]
            in1 = bass.AP(tensor=ib.tensor, offset=ib.offset,
                          ap=[ib.ap[0], [0, KH], ib.ap[1]])
            for h, xT in enumerate(xTs):
                xq = (xqpoolA if h == 0 else xqpoolB).tile([P, KH, P], FP8, tag=f"xq{h}")
                eng = nc.vector if h == 0 else nc.gpsimd
                eng.tensor_tensor(out=xq[:], in0=xT[:], in1=in1,
                                  op=mybir.AluOpType.mult)
                xq_half[mt][h] = xq

        # ---- preamble ----
        # The scheduler serializes every DMA-transpose against the most
        # recently issued regular DMA (and vice versa), so transposes go
        # FIRST -- before w0 -- or tr0 stalls behind w0's 16us transfer.
        # All 16 XBAR transposes are issued up front (paced by the deep xT
        # pool WAR deps at quantize speed); weight slabs stay whole on the
        # scalar ring, woven between transpose groups.
        issue_transposes(0)
        issue_wslab(0, split=False)
        issue_transposes(1)
        issue_transposes(2)
        issue_wslab(1, split=False)
        issue_wsb(0)
        issue_wsb(1)
        for mt in range(3, M_TILES):
            issue_transposes(mt)
        issue_chain(0)
        issue_chain(1)
        issue_chain(2)

        # ---- main GEMM: rectangle-grow order over (nb, mt) ----
        order = _gemm_order()
        pre_slot = {0: [("c", 3)], 2: [("c", 4)], 4: [("c", 5), ("w", 2)],
                    6: [("c", 6)], 8: [("c", 7), ("b", 2)],
                    14: [("w", 3)], 16: [("b", 3)], 24: [("w", 4), ("b", 4)],
                    33: [("w", 5), ("b", 5)], 41: [("w", 6), ("b", 6)],
                    49: [("w", 7), ("b", 7)]}
        wsb_done = {0, 1}
        for s, (nb, mt) in enumerate(order):
            for kind, idx in pre_slot.get(s, []):
                if kind == "c":
                    issue_chain(idx)
                elif kind == "w":
                    issue_wslab(idx, split=False)
                else:
                    issue_wsb(idx)
            wslab = wslab_tiles[nb]
            pm = psum_mm.tile([P, N_BLK], F32, tag="pm")
            for j in range(K_SUPERS):
                h, jj = divmod(j, 8)
                nc.tensor.matmul(
                    out=pm[:],
                    lhsT=xq_half[mt][h][:, 2 * jj:2 * jj + 2, :],
                    rhs=wslab[:, 2 * j:2 * j + 2, :],
                    start=(j == 0), stop=(j == K_SUPERS - 1),
                    perf_mode=mybir.MatmulPerfMode.DoubleRow,
                )
            sb = opool.tile([P, N_BLK], BF16, tag="sb")
            nc.vector.scalar_tensor_tensor(
                out=sb[:], in0=pm[:], scalar=xs_tiles[mt][:],
                in1=wsb_tiles[nb][:],
                op0=mybir.AluOpType.mult, op1=mybir.AluOpType.mult,
            )
            nc.gpsimd.tensor_tensor(out=sb[:], in0=sb[:],
                                    in1=bias_tiles[nb][:],
                                    op=mybir.AluOpType.add)
            nc.sync.dma_start(
                out=out_ap[mt * P:(mt + 1) * P, nb * N_BLK:(nb + 1) * N_BLK],
                in_=sb[:],
            )

    nc.compile()
    return nc


def _get_program():
    if "nc" not in _PROGRAM_CACHE:
        _PROGRAM_CACHE["nc"] = _build_program()
    return _PROGRAM_CACHE["nc"]


def _run_sharded(x, weight, weight_scales, bias, trace=False):
    x = np.asarray(x).astype(ml_dtypes.bfloat16, copy=False)
    weight = np.asarray(weight, dtype=np.float32)
    weight_scales = np.asarray(weight_scales, dtype=np.float32)
    bias16 = np.asarray(bias, dtype=np.float32).astype(ml_dtypes.bfloat16)

    # host-side sharding / layout only (lossless fp8 re-encode of weights)
    wt = np.ascontiguousarray(
        weight.T.reshape(K_SUBS, P, N_BLKS, N_BLK).transpose(2, 1, 0, 3)
    ).astype(ml_dtypes.float8_e4m3)
    in_maps = []
    for c in range(NCORES):
        in_maps.append({
            "x": np.ascontiguousarray(x[c * M_SHARD:(c + 1) * M_SHARD]),
            "wt": wt,
            "ws": weight_scales,
            "bias": bias16,
        })

    nc = _get_program()
    res = run_bass_kernel_spmd(nc, in_maps, core_ids=list(range(NCORES)), trace=trace)
    out = np.concatenate([res.results[c]["out"] for c in range(NCORES)], axis=0)
    return out, res.exec_time_ns


def kernel(x, weight, weight_scales, bias):
    out, _ = _run_sharded(x, weight, weight_scales, bias,
                          trace=bool(os.environ.get("KERNEL_TRACE")))
    return out


# revision 16
# speedup vs baseline: 1.3097x; 1.1273x over previous
"""Fp8 per-token/per-channel quantized linear for Trainium2, 8 NeuronCores.

Computation (matches the jax reference):
    amax[m]  = max_k |x[m, k]|                       (x is bf16)
    xs[m]    = max(amax, 1e-10) / 448
    x_q      = e4m3fn_round(x / xs)                  (values up to +-448)
    out      = bf16((x_q @ W^T) * xs * w_scales) + bf16(bias)

Mapping to TRN2 hardware:
  * TRN's fp8 E4M3 saturates at +-240 (256..448 are Inf/NaN), so we quantize
    at HALF scale: x_q' = e4m3_round(x * (224/amax)) == x_q / 2 exactly (the
    fp8 grid is self-similar under powers of two), and fold the factor 2 into
    the output scale: out = psum * (amax/224) * w_scales.  The reference
    weights are already exactly fp8-representable, so casting them is lossless.
  * Sharding: row-parallel over M (8 cores x 1024 rows).  Each core quantizes
    only its own rows and streams the full weight, transposed on host to
    [K, N] tile layout and losslessly re-encoded to fp8.
  * x_q is transposed on-chip into [K, M] layout with PE identity matmuls.
  * Per-ring DMA bandwidth is ~125 GB/s and there are only two HWDGE rings
    (SP + ACT), so x tiles load as halves split across both rings, w-slabs
    stream on the ACT ring, and ws/bias are loaded as per-512-column-block
    broadcasts just in time instead of full up-front broadcasts.
  * The GEMM walks (nb, mt) in a rectangle-grow order (mt-biased 4:1) so the
    early phase only needs a small corner of x-tiles x w-slabs -- this
    removes the x-starvation stalls the plain nb-outer order suffers while
    the quantize pipeline is still filling.
  * Output stage fused: one DVE scalar_tensor_tensor does (psum*xs)*ws ->
    bf16, GpSimd adds the host-precast bf16 bias, SP-ring DMA stores.
  * Main GEMM runs in fp8 with perf_mode=DoubleRow (k=256 per matmul).
"""

import os
import numpy as np
import ml_dtypes
from contextlib import ExitStack

import concourse.bass as bass
import concourse.bacc as bacc
import concourse.tile as tile
from concourse import mybir
from concourse.bass_utils import run_bass_kernel_spmd
from concourse.masks import make_identity

P = 128
M, K, N = 8192, 4096, 4096
NCORES = 8
M_SHARD = M // NCORES          # 1024 rows of x per core
M_TILES = M_SHARD // P         # 8
K_SUBS = K // P                # 32
K_SUPERS = K // (2 * P)        # 16 (DoubleRow consumes 256 rows of K)
N_BLK = 512
N_BLKS = N // N_BLK            # 8

FP8 = mybir.dt.float8e4
F32 = mybir.dt.float32
BF16 = mybir.dt.bfloat16

_PROGRAM_CACHE = {}


def _gemm_order():
    """Rectangle-grow (staircase) enumeration of (nb, mt), mt-biased 4:1."""
    order = [(0, 0)]
    nm, nn = 1, 1
    while nm < M_TILES or nn < N_BLKS:
        if nm < M_TILES and (nm < 4 * nn or nn == N_BLKS):
            order.extend((nb, nm) for nb in range(nn))
            nm += 1
        else:
            order.extend((nn, mt) for mt in range(nm))
            nn += 1
    return order


def _build_program():
    nc = bacc.Bacc(None, target_bir_lowering=False)

    x_d = nc.declare_dram_parameter("x", [M_SHARD, K], BF16, isOutput=False)
    # host layout: wt[nb, p, ksub, n] = weight[nb*512 + n, ksub*128 + p], fp8
    wt_d = nc.declare_dram_parameter("wt", [N_BLKS, P, K_SUBS, N_BLK], FP8, isOutput=False)
    ws_d = nc.declare_dram_parameter("ws", [N], F32, isOutput=False)
    bias_d = nc.declare_dram_parameter("bias", [N], BF16, isOutput=False)
    out_d = nc.declare_dram_parameter("out", [M_SHARD, N], BF16, isOutput=True)

    x_ap = x_d[:]
    wt_ap = wt_d[:]
    out_ap = out_d[:]

    with tile.TileContext(nc) as tc, ExitStack() as ctx:
        singles = ctx.enter_context(tc.tile_pool(name="singles", bufs=1))
        xpool = ctx.enter_context(tc.tile_pool(name="xpool", bufs=4))
        stats = ctx.enter_context(tc.tile_pool(name="stats", bufs=4))
        xspool = ctx.enter_context(tc.tile_pool(name="xspool", bufs=M_TILES))
        xqtpool = ctx.enter_context(tc.tile_pool(name="xqtpool", bufs=M_TILES))
        wpool = ctx.enter_context(tc.tile_pool(name="wpool", bufs=4))
        wsbpool = ctx.enter_context(tc.tile_pool(name="wsbpool", bufs=4))
        biaspool = ctx.enter_context(tc.tile_pool(name="biaspool", bufs=4))
        opool = ctx.enter_context(tc.tile_pool(name="opool", bufs=8))
        psum_tr = ctx.enter_context(tc.tile_pool(name="psum_tr", bufs=2, space="PSUM"))
        psum_mm = ctx.enter_context(tc.tile_pool(name="psum_mm", bufs=4, space="PSUM"))

        wslab_tiles = [None] * N_BLKS
        wsb_tiles = [None] * N_BLKS
        bias_tiles = [None] * N_BLKS
        xs_tiles = [None] * M_TILES
        xqt_tiles = [None] * M_TILES
        mt_copy_insts = [None] * M_TILES
        state = {"prev_inv": None, "done": []}

        def issue_wslab(nb):
            t = wpool.tile([P, K_SUBS, N_BLK], FP8, tag="w")
            nc.scalar.dma_start(out=t[:], in_=wt_ap[nb])
            wslab_tiles[nb] = t

        def issue_wsb(nb):
            w = wsbpool.tile([P, N_BLK], F32, tag="wsb")
            nc.scalar.dma_start(
                out=w[:],
                in_=bass.AP(tensor=ws_d[:].tensor, offset=nb * N_BLK,
                            ap=[[0, P], [1, N_BLK]]),
            )
            wsb_tiles[nb] = w
            b = biaspool.tile([P, N_BLK], BF16, tag="biasb")
            nc.scalar.dma_start(
                out=b[:],
                in_=bass.AP(tensor=bias_d[:].tensor, offset=nb * N_BLK,
                            ap=[[0, P], [1, N_BLK]]),
            )
            bias_tiles[nb] = b

        ident = singles.tile([P, P], FP8)
        make_identity(nc, ident)

        def issue_chain(mt):
            # x row tile, halves split across both DMA rings
            xt = xpool.tile([P, K], BF16, tag="xt")
            r0, r1 = (nc.sync, nc.scalar) if mt % 2 == 0 else (nc.scalar, nc.sync)
            r0.dma_start(out=xt[:, 0:K // 2], in_=x_ap[mt * P:(mt + 1) * P, 0:K // 2])
            r1.dma_start(out=xt[:, K // 2:], in_=x_ap[mt * P:(mt + 1) * P, K // 2:])

            amax = stats.tile([P, 1], F32, tag="amax")
            reduce_inst = nc.vector.tensor_reduce(
                out=amax[:], in_=xt[:],
                axis=mybir.AxisListType.X, op=mybir.AluOpType.max,
                apply_absolute_value=True,
            )
            # DVE order: don't let this reduce jump ahead of the previous
            # tile's tiny scale chain or of older psum eviction copies
            if state["prev_inv"] is not None:
                tile.add_dep_helper(reduce_inst.ins, state["prev_inv"].ins, sync=False,
                                    reason="stats chain before next reduce")
            if len(state["done"]) >= 2:
                for ci in mt_copy_insts[state["done"][-2]]:
                    tile.add_dep_helper(reduce_inst.ins, ci.ins, sync=False,
                                        reason="evict copies before later reduce")
            with tc.high_priority():
                xs = xspool.tile([P, 1], F32, tag="xs")
                nc.vector.tensor_scalar(
                    out=xs[:], in0=amax[:],
                    scalar1=1e-10, scalar2=1.0 / 224.0,
                    op0=mybir.AluOpType.max, op1=mybir.AluOpType.mult,
                )
                xs_tiles[mt] = xs
                inv = stats.tile([P, 1], F32, tag="inv")
                state["prev_inv"] = nc.vector.reciprocal(out=inv[:], in_=xs[:])

            xq = xpool.tile([P, K], FP8, tag="xq")
            nc.scalar.activation(
                out=xq[:], in_=xt[:],
                func=mybir.ActivationFunctionType.Copy, scale=inv[:],
            )

            # transpose x_q into [K, M] layout via PE identity matmuls
            xqt_groups = []
            copy_insts = []
            for q8 in range(K_SUBS // 8):
                xqt_g = xqtpool.tile([P, 8, P], FP8, tag=f"xqt{q8}")
                xqt_groups.append(xqt_g)
                ptr = psum_tr.tile([P, 8, P], F32, tag="ptr")
                for i in range(8):
                    ks = q8 * 8 + i
                    nc.tensor.matmul(
                        out=ptr[:, i, :],
                        lhsT=xq[:, ks * P:(ks + 1) * P],
                        rhs=ident[:],
                        start=True, stop=True,
                    )
                if q8 % 2 == 0:
                    copy_insts.append(nc.scalar.copy(out=xqt_g[:], in_=ptr[:]))
                else:
                    copy_insts.append(nc.vector.tensor_copy(out=xqt_g[:], in_=ptr[:]))
            xqt_tiles[mt] = xqt_groups
            mt_copy_insts[mt] = copy_insts
            state["done"].append(mt)

        # ---- preamble ----
        issue_wslab(0)
        issue_wsb(0)
        issue_chain(0)
        issue_wslab(1)
        issue_chain(1)
        issue_wsb(1)
        issue_chain(2)

        # ---- main fp8 DoubleRow GEMM, rectangle-grow order ----
        order = _gemm_order()
        pre_slot = {2: [("c", 3)], 4: [("c", 4), ("w", 2)],
                    6: [("c", 5)], 8: [("c", 6), ("b", 2)], 10: [("c", 7)],
                    14: [("w", 3)], 16: [("b", 3)], 24: [("w", 4), ("b", 4)],
                    33: [("w", 5), ("b", 5)], 41: [("w", 6), ("b", 6)],
                    49: [("w", 7), ("b", 7)]}
        for s, (nb, mt) in enumerate(order):
            for kind, idx in pre_slot.get(s, []):
                if kind == "c":
                    issue_chain(idx)
                elif kind == "w":
                    issue_wslab(idx)
                else:
                    issue_wsb(idx)
            wslab = wslab_tiles[nb]
            pm = psum_mm.tile([P, N_BLK], F32, tag="pm")
            for j in range(K_SUPERS):
                g, jj = divmod(j, 4)
                nc.tensor.matmul(
                    out=pm[:],
                    lhsT=xqt_tiles[mt][g][:, 2 * jj:2 * jj + 2, :],
                    rhs=wslab[:, 2 * j:2 * j + 2, :],
                    start=(j == 0), stop=(j == K_SUPERS - 1),
                    perf_mode=mybir.MatmulPerfMode.DoubleRow,
                )
            sb = opool.tile([P, N_BLK], BF16, tag="sb")
            nc.vector.scalar_tensor_tensor(
                out=sb[:], in0=pm[:], scalar=xs_tiles[mt][:],
                in1=wsb_tiles[nb][:],
                op0=mybir.AluOpType.mult, op1=mybir.AluOpType.mult,
            )
            nc.gpsimd.tensor_tensor(out=sb[:], in0=sb[:],
                                    in1=bias_tiles[nb][:],
                                    op=mybir.AluOpType.add)
            nc.sync.dma_start(
                out=out_ap[mt * P:(mt + 1) * P, nb * N_BLK:(nb + 1) * N_BLK],
                in_=sb[:],
            )

    nc.compile()
    return nc


def _get_program():
    if "nc" not in _PROGRAM_CACHE:
        _PROGRAM_CACHE["nc"] = _build_program()
    return _PROGRAM_CACHE["nc"]


def _run_sharded(x, weight, weight_scales, bias, trace=False):
    x = np.asarray(x).astype(ml_dtypes.bfloat16, copy=False)
    weight = np.asarray(weight, dtype=np.float32)
    weight_scales = np.asarray(weight_scales, dtype=np.float32)
    bias16 = np.asarray(bias, dtype=np.float32).astype(ml_dtypes.bfloat16)

    # host-side sharding / layout only (lossless fp8 re-encode of weights)
    wt = np.ascontiguousarray(
        weight.T.reshape(K_SUBS, P, N_BLKS, N_BLK).transpose(2, 1, 0, 3)
    ).astype(ml_dtypes.float8_e4m3)
    in_maps = []
    for c in range(NCORES):
        in_maps.append({
            "x": np.ascontiguousarray(x[c * M_SHARD:(c + 1) * M_SHARD]),
            "wt": wt,
            "ws": weight_scales,
            "bias": bias16,
        })

    nc = _get_program()
    res = run_bass_kernel_spmd(nc, in_maps, core_ids=list(range(NCORES)), trace=trace)
    out = np.concatenate([res.results[c]["out"] for c in range(NCORES)], axis=0)
    return out, res.exec_time_ns


def kernel(x, weight, weight_scales, bias):
    out, _ = _run_sharded(x, weight, weight_scales, bias,
                          trace=bool(os.environ.get("KERNEL_TRACE")))
    return out


# revision 17
# speedup vs baseline: 1.3874x; 1.0594x over previous
"""Fp8 per-token/per-channel quantized linear for Trainium2, 8 NeuronCores.

Computation (matches the jax reference):
    amax[m]  = max_k |x[m, k]|                       (x is bf16)
    xs[m]    = max(amax, 1e-10) / 448
    x_q      = e4m3fn_round(x / xs)                  (values up to +-448)
    out      = bf16((x_q @ W^T) * xs * w_scales) + bf16(bias)

Mapping to TRN2 hardware:
  * TRN's fp8 E4M3 saturates at +-240 (256..448 are Inf/NaN), so we quantize
    at HALF scale: x_q' = e4m3_round(x * (224/amax)) == x_q / 2 exactly (the
    fp8 grid is self-similar under powers of two), and fold the factor 2 into
    the output scale: out = psum * (amax/224) * w_scales.  The reference
    weights are already exactly fp8-representable, so casting them is lossless.
  * Sharding: row-parallel over M (8 cores x 1024 rows).  Each core quantizes
    only its own rows (the amax reduction is the expensive vector-engine op;
    replicating it 8x under column-parallel would make the kernel DVE-bound),
    and streams the full weight, transposed on host to [K, N] tile layout and
    losslessly re-encoded to fp8 (4x less HBM traffic than fp32).
  * x_q is transposed on-chip into [K, M] layout with PE identity matmuls
    (contraction must sit on partitions for both matmul operands).
  * Main GEMM runs in fp8 with perf_mode=DoubleRow (k=256 per matmul).
  * Preamble: wslab0 is prefetched ahead of the ws/bias broadcasts on the
    ACT ring (it gates the first main matmul), and the first x tile loads as
    two halves split across both DMA rings to start the amax chain sooner.
"""

import os
import numpy as np
import ml_dtypes
from contextlib import ExitStack

import concourse.bass as bass
import concourse.bacc as bacc
import concourse.tile as tile
from concourse import mybir
from concourse.bass_utils import run_bass_kernel_spmd
from concourse.masks import make_identity

P = 128
M, K, N = 8192, 4096, 4096
NCORES = 8
M_SHARD = M // NCORES          # 1024 rows of x per core
M_TILES = M_SHARD // P         # 8
K_SUBS = K // P                # 32
K_SUPERS = K // (2 * P)        # 16 (DoubleRow consumes 256 rows of K)
N_BLK = 512
N_BLKS = N // N_BLK            # 8

FP8 = mybir.dt.float8e4
F32 = mybir.dt.float32
BF16 = mybir.dt.bfloat16

_PROGRAM_CACHE = {}


def _build_program():
    nc = bacc.Bacc(None, target_bir_lowering=False)

    x_d = nc.declare_dram_parameter("x", [M_SHARD, K], BF16, isOutput=False)
    # host layout: wt[nb, p, ksub, n] = weight[nb*512 + n, ksub*128 + p],
    # losslessly re-encoded to fp8 (reference weights are fp8-round-tripped,
    # i.e. every value is exactly representable in e4m3)
    wt_d = nc.declare_dram_parameter("wt", [N_BLKS, P, K_SUBS, N_BLK], FP8, isOutput=False)
    ws_d = nc.declare_dram_parameter("ws", [N], F32, isOutput=False)
    bias_d = nc.declare_dram_parameter("bias", [N], F32, isOutput=False)
    out_d = nc.declare_dram_parameter("out", [M_SHARD, N], BF16, isOutput=True)

    x_ap = x_d[:]
    wt_ap = wt_d[:]
    out_ap = out_d[:]

    with tile.TileContext(nc) as tc, ExitStack() as ctx:
        singles = ctx.enter_context(tc.tile_pool(name="singles", bufs=1))
        xpool = ctx.enter_context(tc.tile_pool(name="xpool", bufs=4))
        stats = ctx.enter_context(tc.tile_pool(name="stats", bufs=4))
        xspool = ctx.enter_context(tc.tile_pool(name="xspool", bufs=M_TILES))
        xqtpool = ctx.enter_context(tc.tile_pool(name="xqtpool", bufs=M_TILES))
        wpool = ctx.enter_context(tc.tile_pool(name="wpool", bufs=4))
        opool = ctx.enter_context(tc.tile_pool(name="opool", bufs=6))
        psum_tr = ctx.enter_context(tc.tile_pool(name="psum_tr", bufs=2, space="PSUM"))
        psum_mm = ctx.enter_context(tc.tile_pool(name="psum_mm", bufs=4, space="PSUM"))

        # prefetch the first weight slab before anything else on the ACT ring
        # (it gates the first main-GEMM matmul)
        wslab_tiles = [None] * N_BLKS

        def issue_wslab(nb):
            t = wpool.tile([P, K_SUBS, N_BLK], FP8, tag="w")
            nc.scalar.dma_start(out=t[:], in_=wt_ap[nb])
            wslab_tiles[nb] = t

        issue_wslab(0)

        # w-scale / bias broadcasts ride the ACT HWDGE ring so they don't
        # delay the x loads on the sync ring
        ws_b = singles.tile([P, N], F32)
        nc.scalar.dma_start(
            out=ws_b[:],
            in_=bass.AP(tensor=ws_d[:].tensor, offset=0, ap=[[0, P], [1, N]]),
        )
        bias_f32 = singles.tile([P, N], F32)
        nc.scalar.dma_start(
            out=bias_f32[:],
            in_=bass.AP(tensor=bias_d[:].tensor, offset=0, ap=[[0, P], [1, N]]),
        )

        issue_wslab(1)
        issue_wslab(2)
        issue_wslab(3)

        ident = singles.tile([P, P], FP8)
        make_identity(nc, ident)

        bias_b = singles.tile([P, N], BF16)
        nc.gpsimd.tensor_copy(out=bias_b[:], in_=bias_f32[:])

        # ---- quantization phase: per 128-row tile of x ----
        xs_tiles = []
        xqt_tiles = []
        mt_copy_insts = []
        prev_inv_inst = None
        for mt in range(M_TILES):
            xt = xpool.tile([P, K], BF16, tag="xt")
            if mt == 0:
                # first tile: halves on both rings so amax starts sooner
                nc.sync.dma_start(out=xt[:, 0:K // 2],
                                  in_=x_ap[mt * P:(mt + 1) * P, 0:K // 2])
                nc.scalar.dma_start(out=xt[:, K // 2:],
                                    in_=x_ap[mt * P:(mt + 1) * P, K // 2:])
            else:
                nc.sync.dma_start(out=xt[:], in_=x_ap[mt * P:(mt + 1) * P, :])

            amax = stats.tile([P, 1], F32, tag="amax")
            reduce_inst = nc.vector.tensor_reduce(
                out=amax[:], in_=xt[:],
                axis=mybir.AxisListType.X, op=mybir.AluOpType.max,
                apply_absolute_value=True,
            )
            # DVE order: don't let the next tile's reduce jump ahead of this
            # tile's tiny scale chain (it gates the PE via the ACT quant)
            if prev_inv_inst is not None:
                tile.add_dep_helper(reduce_inst.ins, prev_inv_inst.ins, sync=False,
                                    reason="stats chain before next reduce")
            # ... and don't let it jump ahead of the 2-tiles-ago psum
            # eviction copies (they unblock the PE transposes)
            if mt >= 2:
                for ci in mt_copy_insts[mt - 2]:
                    tile.add_dep_helper(reduce_inst.ins, ci.ins, sync=False,
                                        reason="evict copies before later reduce")
            with tc.high_priority():
                # xs = max(amax, eps) * (1/224); quant scale is exactly 1/xs
                xs = xspool.tile([P, 1], F32, tag="xs")
                nc.vector.tensor_scalar(
                    out=xs[:], in0=amax[:],
                    scalar1=1e-10, scalar2=1.0 / 224.0,
                    op0=mybir.AluOpType.max, op1=mybir.AluOpType.mult,
                )
                xs_tiles.append(xs)
                inv = stats.tile([P, 1], F32, tag="inv")
                prev_inv_inst = nc.vector.reciprocal(out=inv[:], in_=xs[:])

            xq = xpool.tile([P, K], FP8, tag="xq")
            nc.scalar.activation(
                out=xq[:], in_=xt[:],
                func=mybir.ActivationFunctionType.Copy, scale=inv[:],
            )

            # transpose x_q into [K, M] layout via PE identity matmuls
            xqt_groups = []
            copy_insts = []
            for q8 in range(K_SUBS // 8):
                xqt_g = xqtpool.tile([P, 8, P], FP8, tag=f"xqt{q8}")
                xqt_groups.append(xqt_g)
                ptr = psum_tr.tile([P, 8, P], F32, tag="ptr")
                for i in range(8):
                    ks = q8 * 8 + i
                    nc.tensor.matmul(
                        out=ptr[:, i, :],
                        lhsT=xq[:, ks * P:(ks + 1) * P],
                        rhs=ident[:],
                        start=True, stop=True,
                    )
                if q8 % 2 == 0:
                    copy_insts.append(nc.scalar.copy(out=xqt_g[:], in_=ptr[:]))
                else:
                    copy_insts.append(nc.vector.tensor_copy(out=xqt_g[:], in_=ptr[:]))
            xqt_tiles.append(xqt_groups)
            mt_copy_insts.append(copy_insts)

        # ---- main fp8 DoubleRow GEMM, streamed over 512-col blocks of N ----
        for nb in range(N_BLKS):
            if nb + 4 < N_BLKS:
                issue_wslab(nb + 4)
            wslab = wslab_tiles[nb]

            for mt in range(M_TILES):
                pm = psum_mm.tile([P, N_BLK], F32, tag="pm")
                for j in range(K_SUPERS):
                    g, jj = divmod(j, 4)
                    nc.tensor.matmul(
                        out=pm[:],
                        lhsT=xqt_tiles[mt][g][:, 2 * jj:2 * jj + 2, :],
                        rhs=wslab[:, 2 * j:2 * j + 2, :],
                        start=(j == 0), stop=(j == K_SUPERS - 1),
                        perf_mode=mybir.MatmulPerfMode.DoubleRow,
                    )
                sb1 = opool.tile([P, N_BLK], F32, tag="sb1")
                nc.scalar.activation(
                    out=sb1[:], in_=pm[:],
                    func=mybir.ActivationFunctionType.Copy, scale=xs_tiles[mt][:],
                )
                sb2 = opool.tile([P, N_BLK], BF16, tag="sb2")
                nc.vector.tensor_mul(sb2[:], sb1[:], ws_b[:, nb * N_BLK:(nb + 1) * N_BLK])
                nc.vector.tensor_add(sb2[:], sb2[:], bias_b[:, nb * N_BLK:(nb + 1) * N_BLK])
                nc.sync.dma_start(
                    out=out_ap[mt * P:(mt + 1) * P, nb * N_BLK:(nb + 1) * N_BLK],
                    in_=sb2[:],
                )

    nc.compile()
    return nc


def _get_program():
    if "nc" not in _PROGRAM_CACHE:
        _PROGRAM_CACHE["nc"] = _build_program()
    return _PROGRAM_CACHE["nc"]


def _run_sharded(x, weight, weight_scales, bias, trace=False):
    x = np.asarray(x).astype(ml_dtypes.bfloat16, copy=False)
    weight = np.asarray(weight, dtype=np.float32)
    weight_scales = np.asarray(weight_scales, dtype=np.float32)
    bias = np.asarray(bias, dtype=np.float32)

    # host-side sharding / layout only:
    # wt[nb, p, ksub, n] = weight[nb*512 + n, ksub*128 + p], re-encoded to
    # fp8 e4m3 (lossless: the reference weights are fp8-round-tripped values)
    wt = np.ascontiguousarray(
        weight.T.reshape(K_SUBS, P, N_BLKS, N_BLK).transpose(2, 1, 0, 3)
    ).astype(ml_dtypes.float8_e4m3)
    in_maps = []
    for c in range(NCORES):
        in_maps.append({
            "x": np.ascontiguousarray(x[c * M_SHARD:(c + 1) * M_SHARD]),
            "wt": wt,
            "ws": weight_scales,
            "bias": bias,
        })

    nc = _get_program()
    res = run_bass_kernel_spmd(nc, in_maps, core_ids=list(range(NCORES)), trace=trace)
    out = np.concatenate([res.results[c]["out"] for c in range(NCORES)], axis=0)
    return out, res.exec_time_ns


def kernel(x, weight, weight_scales, bias):
    out, _ = _run_sharded(x, weight, weight_scales, bias,
                          trace=bool(os.environ.get("KERNEL_TRACE")))
    return out


# revision 18
# speedup vs baseline: 1.5572x; 1.1224x over previous
"""Fp8 per-token/per-channel quantized linear for Trainium2, 8 NeuronCores.

Computation (matches the jax reference):
    amax[m]  = max_k |x[m, k]|                       (x is bf16)
    xs[m]    = max(amax, 1e-10) / 448
    x_q      = e4m3fn_round(x / xs)                  (values up to +-448)
    out      = bf16((x_q @ W^T) * xs * w_scales) + bf16(bias)

Mapping to TRN2 hardware:
  * TRN's fp8 E4M3 saturates at +-240 (256..448 are Inf/NaN), so we quantize
    at HALF scale: x_q' = e4m3_round(x * (224/amax)) == x_q / 2 exactly (the
    fp8 grid is self-similar under powers of two), and fold the factor 2 into
    the output scale: out = psum * (amax/224) * w_scales.  The reference
    weights are already exactly fp8-representable, so casting them is lossless.
  * Sharding: row-parallel over M (8 cores x 1024 rows).  Each core quantizes
    only its own rows (the amax reduction is the expensive vector-engine op;
    replicating it 8x under column-parallel would make the kernel DVE-bound),
    and streams the full weight, transposed on host to [K, N] tile layout and
    losslessly re-encoded to fp8 (4x less HBM traffic than fp32).
  * x_q is transposed on-chip into [K, M] layout with PE identity matmuls
    (contraction must sit on partitions for both matmul operands).
  * Main GEMM runs in fp8 with perf_mode=DoubleRow (k=256 per matmul).
  * Preamble: wslab0 is prefetched ahead of the ws/bias broadcasts on the
    ACT ring (it gates the first main matmul), and the first x tile loads as
    two halves split across both DMA rings to start the amax chain sooner.
"""

import os
import numpy as np
import ml_dtypes
from contextlib import ExitStack

import concourse.bass as bass
import concourse.bacc as bacc
import concourse.tile as tile
from concourse import mybir
from concourse.bass_utils import run_bass_kernel_spmd
from concourse.masks import make_identity

P = 128
M, K, N = 8192, 4096, 4096
NCORES = 8
M_SHARD = M // NCORES          # 1024 rows of x per core
M_TILES = M_SHARD // P         # 8
K_SUBS = K // P                # 32
K_SUPERS = K // (2 * P)        # 16 (DoubleRow consumes 256 rows of K)
N_BLK = 512
N_BLKS = N // N_BLK            # 8

FP8 = mybir.dt.float8e4
F32 = mybir.dt.float32
BF16 = mybir.dt.bfloat16

_PROGRAM_CACHE = {}


def _build_program():
    nc = bacc.Bacc(None, target_bir_lowering=False)

    x_d = nc.declare_dram_parameter("x", [M_SHARD, K], BF16, isOutput=False)
    # host layout: wt[nb, p, ksub, n] = weight[nb*512 + n, ksub*128 + p],
    # losslessly re-encoded to fp8 (reference weights are fp8-round-tripped,
    # i.e. every value is exactly representable in e4m3)
    wt_d = nc.declare_dram_parameter("wt", [N_BLKS, P, K_SUBS, N_BLK], FP8, isOutput=False)
    ws_d = nc.declare_dram_parameter("ws", [N], F32, isOutput=False)
    bias_d = nc.declare_dram_parameter("bias", [N], F32, isOutput=False)
    out_d = nc.declare_dram_parameter("out", [M_SHARD, N], BF16, isOutput=True)

    x_ap = x_d[:]
    wt_ap = wt_d[:]
    out_ap = out_d[:]

    with tile.TileContext(nc) as tc, ExitStack() as ctx:
        singles = ctx.enter_context(tc.tile_pool(name="singles", bufs=1))
        xpool = ctx.enter_context(tc.tile_pool(name="xpool", bufs=4))
        stats = ctx.enter_context(tc.tile_pool(name="stats", bufs=4))
        xspool = ctx.enter_context(tc.tile_pool(name="xspool", bufs=M_TILES))
        xqtpool = ctx.enter_context(tc.tile_pool(name="xqtpool", bufs=M_TILES))
        wpool = ctx.enter_context(tc.tile_pool(name="wpool", bufs=4))
        opool = ctx.enter_context(tc.tile_pool(name="opool", bufs=6))
        psum_tr = ctx.enter_context(tc.tile_pool(name="psum_tr", bufs=2, space="PSUM"))
        psum_mm = ctx.enter_context(tc.tile_pool(name="psum_mm", bufs=4, space="PSUM"))

        # prefetch the first weight slab before anything else on the ACT ring
        # (it gates the first main-GEMM matmul)
        wslab_tiles = [None] * N_BLKS

        def issue_wslab(nb):
            t = wpool.tile([P, K_SUBS, N_BLK], FP8, tag="w")
            nc.scalar.dma_start(out=t[:], in_=wt_ap[nb])
            wslab_tiles[nb] = t

        issue_wslab(0)

        # w-scale / bias broadcasts ride the ACT HWDGE ring so they don't
        # delay the x loads on the sync ring
        ws_b = singles.tile([P, N], F32)
        nc.scalar.dma_start(
            out=ws_b[:],
            in_=bass.AP(tensor=ws_d[:].tensor, offset=0, ap=[[0, P], [1, N]]),
        )
        bias_f32 = singles.tile([P, N], F32)
        nc.scalar.dma_start(
            out=bias_f32[:],
            in_=bass.AP(tensor=bias_d[:].tensor, offset=0, ap=[[0, P], [1, N]]),
        )

        issue_wslab(1)
        issue_wslab(2)
        issue_wslab(3)

        ident = singles.tile([P, P], FP8)
        make_identity(nc, ident)

        bias_b = singles.tile([P, N], BF16)
        nc.gpsimd.tensor_copy(out=bias_b[:], in_=bias_f32[:])

        # ---- quantization phase: per 128-row tile of x ----
        xs_tiles = []
        xqt_tiles = []
        mt_copy_insts = []
        prev_inv_inst = None
        for mt in range(M_TILES):
            xt = xpool.tile([P, K], BF16, tag="xt")
            nc.sync.dma_start(out=xt[:], in_=x_ap[mt * P:(mt + 1) * P, :])

            amax = stats.tile([P, 1], F32, tag="amax")
            reduce_inst = nc.vector.tensor_reduce(
                out=amax[:], in_=xt[:],
                axis=mybir.AxisListType.X, op=mybir.AluOpType.max,
                apply_absolute_value=True,
            )
            # DVE order: don't let the next tile's reduce jump ahead of this
            # tile's tiny scale chain (it gates the PE via the ACT quant)
            if prev_inv_inst is not None:
                tile.add_dep_helper(reduce_inst.ins, prev_inv_inst.ins, sync=False,
                                    reason="stats chain before next reduce")
            # ... and don't let it jump ahead of the 2-tiles-ago psum
            # eviction copies (they unblock the PE transposes)
            if mt >= 2:
                for ci in mt_copy_insts[mt - 2]:
                    tile.add_dep_helper(reduce_inst.ins, ci.ins, sync=False,
                                        reason="evict copies before later reduce")
            with tc.high_priority():
                # xs = max(amax, eps) * (1/224); quant scale is exactly 1/xs
                xs = xspool.tile([P, 1], F32, tag="xs")
                nc.vector.tensor_scalar(
                    out=xs[:], in0=amax[:],
                    scalar1=1e-10, scalar2=1.0 / 224.0,
                    op0=mybir.AluOpType.max, op1=mybir.AluOpType.mult,
                )
                xs_tiles.append(xs)
                inv = stats.tile([P, 1], F32, tag="inv")
                prev_inv_inst = nc.vector.reciprocal(out=inv[:], in_=xs[:])

            xq = xpool.tile([P, K], FP8, tag="xq")
            nc.scalar.activation(
                out=xq[:], in_=xt[:],
                func=mybir.ActivationFunctionType.Copy, scale=inv[:],
            )

            # transpose x_q into [K, M] layout via PE identity matmuls
            xqt_groups = []
            copy_insts = []
            for q8 in range(K_SUBS // 8):
                xqt_g = xqtpool.tile([P, 8, P], FP8, tag=f"xqt{q8}")
                xqt_groups.append(xqt_g)
                ptr = psum_tr.tile([P, 8, P], F32, tag="ptr")
                for i in range(8):
                    ks = q8 * 8 + i
                    nc.tensor.matmul(
                        out=ptr[:, i, :],
                        lhsT=xq[:, ks * P:(ks + 1) * P],
                        rhs=ident[:],
                        start=True, stop=True,
                    )
                if q8 % 2 == 0:
                    copy_insts.append(nc.scalar.copy(out=xqt_g[:], in_=ptr[:]))
                else:
                    copy_insts.append(nc.vector.tensor_copy(out=xqt_g[:], in_=ptr[:]))
            xqt_tiles.append(xqt_groups)
            mt_copy_insts.append(copy_insts)

        # ---- main fp8 DoubleRow GEMM, streamed over 512-col blocks of N ----
        for nb in range(N_BLKS):
            if nb + 4 < N_BLKS:
                issue_wslab(nb + 4)
            wslab = wslab_tiles[nb]

            for mt in range(M_TILES):
                pm = psum_mm.tile([P, N_BLK], F32, tag="pm")
                for j in range(K_SUPERS):
                    g, jj = divmod(j, 4)
                    nc.tensor.matmul(
                        out=pm[:],
                        lhsT=xqt_tiles[mt][g][:, 2 * jj:2 * jj + 2, :],
                        rhs=wslab[:, 2 * j:2 * j + 2, :],
                        start=(j == 0), stop=(j == K_SUPERS - 1),
                        perf_mode=mybir.MatmulPerfMode.DoubleRow,
                    )
                sb1 = opool.tile([P, N_BLK], F32, tag="sb1")
                nc.scalar.activation(
                    out=sb1[:], in_=pm[:],
                    func=mybir.ActivationFunctionType.Copy, scale=xs_tiles[mt][:],
                )
                sb2 = opool.tile([P, N_BLK], BF16, tag="sb2")
                nc.vector.tensor_mul(sb2[:], sb1[:], ws_b[:, nb * N_BLK:(nb + 1) * N_BLK])
                nc.vector.tensor_add(sb2[:], sb2[:], bias_b[:, nb * N_BLK:(nb + 1) * N_BLK])
                nc.sync.dma_start(
                    out=out_ap[mt * P:(mt + 1) * P, nb * N_BLK:(nb + 1) * N_BLK],
                    in_=sb2[:],
                )

    nc.compile()
    return nc


def _get_program():
    if "nc" not in _PROGRAM_CACHE:
        _PROGRAM_CACHE["nc"] = _build_program()
    return _PROGRAM_CACHE["nc"]


def _run_sharded(x, weight, weight_scales, bias, trace=False):
    x = np.asarray(x).astype(ml_dtypes.bfloat16, copy=False)
    weight = np.asarray(weight, dtype=np.float32)
    weight_scales = np.asarray(weight_scales, dtype=np.float32)
    bias = np.asarray(bias, dtype=np.float32)

    # host-side sharding / layout only:
    # wt[nb, p, ksub, n] = weight[nb*512 + n, ksub*128 + p], re-encoded to
    # fp8 e4m3 (lossless: the reference weights are fp8-round-tripped values)
    wt = np.ascontiguousarray(
        weight.T.reshape(K_SUBS, P, N_BLKS, N_BLK).transpose(2, 1, 0, 3)
    ).astype(ml_dtypes.float8_e4m3)
    in_maps = []
    for c in range(NCORES):
        in_maps.append({
            "x": np.ascontiguousarray(x[c * M_SHARD:(c + 1) * M_SHARD]),
            "wt": wt,
            "ws": weight_scales,
            "bias": bias,
        })

    nc = _get_program()
    res = run_bass_kernel_spmd(nc, in_maps, core_ids=list(range(NCORES)), trace=trace)
    out = np.concatenate([res.results[c]["out"] for c in range(NCORES)], axis=0)
    return out, res.exec_time_ns


def kernel(x, weight, weight_scales, bias):
    out, _ = _run_sharded(x, weight, weight_scales, bias,
                          trace=bool(os.environ.get("KERNEL_TRACE")))
    return out


# revision 20
# speedup vs baseline: 1.6092x; 1.0334x over previous
"""Fp8 per-token/per-channel quantized linear for Trainium2, 8 NeuronCores.

Computation (matches the jax reference):
    amax[m]  = max_k |x[m, k]|                       (x is bf16)
    xs[m]    = max(amax, 1e-10) / 448
    x_q      = e4m3fn_round(x / xs)                  (values up to +-448)
    out      = bf16((x_q @ W^T) * xs * w_scales) + bf16(bias)

Mapping to TRN2 hardware:
  * TRN's fp8 E4M3 saturates at +-240 (256..448 are Inf/NaN), so we quantize
    at HALF scale: x_q' = e4m3_round(x * (224/amax)) == x_q / 2 exactly (the
    fp8 grid is self-similar under powers of two), and fold the factor 2 into
    the output scale: out = psum * (amax/224) * w_scales.  The reference
    weights are already exactly fp8-representable, so casting them is lossless.
  * Sharding: row-parallel over M (8 cores x 1024 rows).  Each core quantizes
    only its own rows (the amax reduction is the expensive vector-engine op;
    replicating it 8x under column-parallel would make the kernel DVE-bound),
    and streams the full weight, transposed on host to [K, N] tile layout and
    losslessly re-encoded to fp8 (4x less HBM traffic than fp32).
  * x_q is transposed on-chip into [K, M] layout with PE identity matmuls
    (contraction must sit on partitions for both matmul operands).
  * Main GEMM runs in fp8 with perf_mode=DoubleRow (k=256 per matmul).
  * Preamble: wslab0 is prefetched ahead of the ws/bias broadcasts on the
    ACT ring (it gates the first main matmul).
"""

import os
import numpy as np
import ml_dtypes
from contextlib import ExitStack

import concourse.bass as bass
import concourse.bacc as bacc
import concourse.tile as tile
from concourse import mybir
from concourse.bass_utils import run_bass_kernel_spmd
from concourse.masks import make_identity

P = 128
M, K, N = 8192, 4096, 4096
NCORES = 8
M_SHARD = M // NCORES          # 1024 rows of x per core
M_TILES = M_SHARD // P         # 8
K_SUBS = K // P                # 32
K_SUPERS = K // (2 * P)        # 16 (DoubleRow consumes 256 rows of K)
N_BLK = 512
N_BLKS = N // N_BLK            # 8

FP8 = mybir.dt.float8e4
F32 = mybir.dt.float32
BF16 = mybir.dt.bfloat16

_PROGRAM_CACHE = {}


def _build_program():
    nc = bacc.Bacc(None, target_bir_lowering=False)

    x_d = nc.declare_dram_parameter("x", [M_SHARD, K], BF16, isOutput=False)
    # host layout: wt[nb, p, ksub, n] = weight[nb*512 + n, ksub*128 + p],
    # losslessly re-encoded to fp8 (reference weights are fp8-round-tripped,
    # i.e. every value is exactly representable in e4m3)
    wt_d = nc.declare_dram_parameter("wt", [N_BLKS, P, K_SUBS, N_BLK], FP8, isOutput=False)
    ws_d = nc.declare_dram_parameter("ws", [N], F32, isOutput=False)
    bias_d = nc.declare_dram_parameter("bias", [N], F32, isOutput=False)
    out_d = nc.declare_dram_parameter("out", [M_SHARD, N], BF16, isOutput=True)

    x_ap = x_d[:]
    wt_ap = wt_d[:]
    out_ap = out_d[:]

    with tile.TileContext(nc) as tc, ExitStack() as ctx:
        singles = ctx.enter_context(tc.tile_pool(name="singles", bufs=1))
        xpool = ctx.enter_context(tc.tile_pool(name="xpool", bufs=4))
        stats = ctx.enter_context(tc.tile_pool(name="stats", bufs=4))
        xspool = ctx.enter_context(tc.tile_pool(name="xspool", bufs=M_TILES))
        xqtpool = ctx.enter_context(tc.tile_pool(name="xqtpool", bufs=M_TILES))
        wpool = ctx.enter_context(tc.tile_pool(name="wpool", bufs=4))
        opool = ctx.enter_context(tc.tile_pool(name="opool", bufs=6))
        psum_tr = ctx.enter_context(tc.tile_pool(name="psum_tr", bufs=2, space="PSUM"))
        psum_mm = ctx.enter_context(tc.tile_pool(name="psum_mm", bufs=4, space="PSUM"))

        # prefetch the first weight slab before anything else on the ACT ring
        # (it gates the first main-GEMM matmul)
        wslab_tiles = [None] * N_BLKS

        def issue_wslab(nb):
            t = wpool.tile([P, K_SUBS, N_BLK], FP8, tag="w")
            nc.scalar.dma_start(out=t[:], in_=wt_ap[nb])
            wslab_tiles[nb] = t

        issue_wslab(0)

        # w-scale / bias broadcasts ride the ACT HWDGE ring so they don't
        # delay the x loads on the sync ring
        ws_b = singles.tile([P, N], F32)
        nc.scalar.dma_start(
            out=ws_b[:],
            in_=bass.AP(tensor=ws_d[:].tensor, offset=0, ap=[[0, P], [1, N]]),
        )
        bias_f32 = singles.tile([P, N], F32)
        nc.scalar.dma_start(
            out=bias_f32[:],
            in_=bass.AP(tensor=bias_d[:].tensor, offset=0, ap=[[0, P], [1, N]]),
        )

        issue_wslab(1)

        ident = singles.tile([P, P], FP8)
        make_identity(nc, ident)

        bias_b = singles.tile([P, N], BF16)
        nc.gpsimd.tensor_copy(out=bias_b[:], in_=bias_f32[:])

        # ---- quantization phase: per 128-row tile of x ----
        xs_tiles = []
        xqt_tiles = []
        mt_copy_insts = []
        prev_inv_inst = None
        for mt in range(M_TILES):
            xt = xpool.tile([P, K], BF16, tag="xt")
            nc.sync.dma_start(out=xt[:], in_=x_ap[mt * P:(mt + 1) * P, :])

            amax = stats.tile([P, 1], F32, tag="amax")
            reduce_inst = nc.vector.tensor_reduce(
                out=amax[:], in_=xt[:],
                axis=mybir.AxisListType.X, op=mybir.AluOpType.max,
                apply_absolute_value=True,
            )
            # DVE order: don't let the next tile's reduce jump ahead of this
            # tile's tiny scale chain (it gates the PE via the ACT quant)
            if prev_inv_inst is not None:
                tile.add_dep_helper(reduce_inst.ins, prev_inv_inst.ins, sync=False,
                                    reason="stats chain before next reduce")
            # ... and don't let it jump ahead of the 2-tiles-ago psum
            # eviction copies (they unblock the PE transposes)
            if mt >= 2:
                for ci in mt_copy_insts[mt - 2]:
                    tile.add_dep_helper(reduce_inst.ins, ci.ins, sync=False,
                                        reason="evict copies before later reduce")
            with tc.high_priority():
                # xs = max(amax, eps) * (1/224); quant scale is exactly 1/xs
                xs = xspool.tile([P, 1], F32, tag="xs")
                nc.vector.tensor_scalar(
                    out=xs[:], in0=amax[:],
                    scalar1=1e-10, scalar2=1.0 / 224.0,
                    op0=mybir.AluOpType.max, op1=mybir.AluOpType.mult,
                )
                xs_tiles.append(xs)
                inv = stats.tile([P, 1], F32, tag="inv")
                prev_inv_inst = nc.vector.reciprocal(out=inv[:], in_=xs[:])

            xq = xpool.tile([P, K], FP8, tag="xq")
            nc.scalar.activation(
                out=xq[:], in_=xt[:],
                func=mybir.ActivationFunctionType.Copy, scale=inv[:],
            )

            # transpose x_q into [K, M] layout via PE identity matmuls
            xqt_groups = []
            copy_insts = []
            for q8 in range(K_SUBS // 8):
                xqt_g = xqtpool.tile([P, 8, P], FP8, tag=f"xqt{q8}")
                xqt_groups.append(xqt_g)
                ptr = psum_tr.tile([P, 8, P], F32, tag="ptr")
                for i in range(8):
                    ks = q8 * 8 + i
                    nc.tensor.matmul(
                        out=ptr[:, i, :],
                        lhsT=xq[:, ks * P:(ks + 1) * P],
                        rhs=ident[:],
                        start=True, stop=True,
                    )
                if q8 % 2 == 0:
                    copy_insts.append(nc.scalar.copy(out=xqt_g[:], in_=ptr[:]))
                else:
                    copy_insts.append(nc.vector.tensor_copy(out=xqt_g[:], in_=ptr[:]))
            xqt_tiles.append(xqt_groups)
            mt_copy_insts.append(copy_insts)
            # wslab2/3 descriptor issues can block the ACT queue on
            # completion-semaphore slot reuse; issue them only after the
            # first chains' ACT quantizes are in the queue
            if mt == 0:
                issue_wslab(2)
            elif mt == 1:
                issue_wslab(3)

        # ---- main fp8 DoubleRow GEMM, streamed over 512-col blocks of N ----
        for nb in range(N_BLKS):
            if nb + 4 < N_BLKS:
                issue_wslab(nb + 4)
            wslab = wslab_tiles[nb]

            for mt in range(M_TILES):
                pm = psum_mm.tile([P, N_BLK], F32, tag="pm")
                for j in range(K_SUPERS):
                    g, jj = divmod(j, 4)
                    nc.tensor.matmul(
                        out=pm[:],
                        lhsT=xqt_tiles[mt][g][:, 2 * jj:2 * jj + 2, :],
                        rhs=wslab[:, 2 * j:2 * j + 2, :],
                        start=(j == 0), stop=(j == K_SUPERS - 1),
                        perf_mode=mybir.MatmulPerfMode.DoubleRow,
                    )
                sb2 = opool.tile([P, N_BLK], BF16, tag="sb2")
                nc.vector.scalar_tensor_tensor(
                    out=sb2[:], in0=pm[:], scalar=xs_tiles[mt][:],
                    in1=ws_b[:, nb * N_BLK:(nb + 1) * N_BLK],
                    op0=mybir.AluOpType.mult, op1=mybir.AluOpType.mult,
                )
                nc.gpsimd.tensor_tensor(out=sb2[:], in0=sb2[:],
                                        in1=bias_b[:, nb * N_BLK:(nb + 1) * N_BLK],
                                        op=mybir.AluOpType.add)
                nc.sync.dma_start(
                    out=out_ap[mt * P:(mt + 1) * P, nb * N_BLK:(nb + 1) * N_BLK],
                    in_=sb2[:],
                )

    nc.compile()
    return nc


def _get_program():
    if "nc" not in _PROGRAM_CACHE:
        _PROGRAM_CACHE["nc"] = _build_program()
    return _PROGRAM_CACHE["nc"]


def _run_sharded(x, weight, weight_scales, bias, trace=False):
    x = np.asarray(x).astype(ml_dtypes.bfloat16, copy=False)
    weight = np.asarray(weight, dtype=np.float32)
    weight_scales = np.asarray(weight_scales, dtype=np.float32)
    bias = np.asarray(bias, dtype=np.float32)

    # host-side sharding / layout only:
    # wt[nb, p, ksub, n] = weight[nb*512 + n, ksub*128 + p], re-encoded to
    # fp8 e4m3 (lossless: the reference weights are fp8-round-tripped values)
    wt = np.ascontiguousarray(
        weight.T.reshape(K_SUBS, P, N_BLKS, N_BLK).transpose(2, 1, 0, 3)
    ).astype(ml_dtypes.float8_e4m3)
    in_maps = []
    for c in range(NCORES):
        in_maps.append({
            "x": np.ascontiguousarray(x[c * M_SHARD:(c + 1) * M_SHARD]),
            "wt": wt,
            "ws": weight_scales,
            "bias": bias,
        })

    nc = _get_program()
    res = run_bass_kernel_spmd(nc, in_maps, core_ids=list(range(NCORES)), trace=trace)
    out = np.concatenate([res.results[c]["out"] for c in range(NCORES)], axis=0)
    return out, res.exec_time_ns


def kernel(x, weight, weight_scales, bias):
    out, _ = _run_sharded(x, weight, weight_scales, bias,
                          trace=bool(os.environ.get("KERNEL_TRACE")))
    return out
